# revision 3
# baseline (speedup 1.0000x reference)
"""Multi-head attention (B=2, L=S=2048, D=1024, H=16) on 8 Trainium2 cores.

Sharding: core c -> batch b = c // 4, head group g = c % 4 (4 heads per core).
W_Q/K/V column-sharded (256 cols per core), W_O row-sharded (256 rows per core);
the 4 partial outputs per batch are summed on the host (plus bias terms).

Per-core pipeline (all big tensors kept transposed so no on-device transposes):
  projections: QT = 0.125*(x Wq + bq)^T, KT = (x Wk + bk)^T (feature-major
    [256, L]); Vaug = [V_h | ones] per head (seq-major, fp16), V bias folded
    out on the host (softmax rows sum to 1 => + bv @ Wo + bo once).
  attention, per (l-tile 512, s-tile 128): S^T = KT^T QT (row-packed pairs of
    heads, K=64); E = exp(S^T) * maskT (ACT exp from PSUM, one fused 0/1 fp16
    mask multiply per s-tile covering both pairs on DVE at 2x); T_h +=
    Vaug_h^T E accumulates BOTH the head output AND its softmax row-sums in
    one full-array matmul (ones columns act as the reducer; even heads get
    [V|1] -> av in rows 0:64, odd heads [1|V] -> av in rows 64:128 so every
    result lands on the lanes the output-projection layout needs). Per
    l-tile: reciprocal_approx_fast on the sum half, DMA lane-swap to the av
    half's partitions, multiply into outT (fp16).
  out-projection: out_partial = outT^T Wo_rows (K=128, accumulate over the
    two 128-row groups).

Software pipelining: the kernel is EXP/PE co-limited, so emission is fully
pipelined: only KT chunk 0 + QT l-chunk 0 + V s-tiles 0-3 are produced up
front (~16us); the remaining KT chunks, V s-tile pairs and QT chunks are
injected in ~1-2us quanta between attention s-tile iterations of l-tile 0,
and each l-tile's output projection is injected into the next l-tile's
s-loop. V/QT/KT/out-proj all borrow the score PSUM slot rotation ("sc" tag)
so the Ts accumulator banks stay live for attention the whole time.

All matmul operands fp16 (1 cyc/row, no packing restrictions); PSUM fp32.
PSUM budget 8 banks = scores 2x2 + T_h 4x1; projections and output-
projection borrow the score slots, so phases overlap without barriers.
"""
from contextlib import ExitStack

import numpy as np

import concourse.bass as bass
import concourse.mybir as mybir
import concourse.tile as tile
from concourse import bacc
from concourse.bass_utils import run_bass_kernel_spmd

F16 = mybir.dt.float16
F32 = mybir.dt.float32

D = 1024          # d_model
H = 16            # heads
DK = 64           # head dim
B, L = 2, 2048
NCORES = 8
HPC = 4           # heads per core
FPC = HPC * DK    # features per core = 256
KD = D // 128     # 8 contraction subtiles for projections
LT, LTW = 4, 512  # l tiles
ST, STW = 16, 128  # s tiles
MPF = 3           # mask DMA prefetch depth
Ident = mybir.ActivationFunctionType.Identity
Exp = mybir.ActivationFunctionType.Exp

_CACHED_NC = None


def _build():
    nc = bacc.Bacc("TRN2", target_bir_lowering=False, debug=False,
                   num_devices=NCORES)
    xT = nc.declare_dram_parameter("xT", [128, KD, L], F16, isOutput=False)
    wq = nc.declare_dram_parameter("wq", [128, KD, FPC], F16, isOutput=False)
    wk = nc.declare_dram_parameter("wk", [128, KD, FPC], F16, isOutput=False)
    wv = nc.declare_dram_parameter("wv", [128, KD, FPC], F16, isOutput=False)
    wo = nc.declare_dram_parameter("wo", [128, 2, D], F16, isOutput=False)
    bq = nc.declare_dram_parameter("bq", [128, 2], F32, isOutput=False)
    bk = nc.declare_dram_parameter("bk", [128, 2], F32, isOutput=False)
    maskT = nc.declare_dram_parameter("maskT", [ST, LT, 128, LTW], F16,
                                      isOutput=False)
    out = nc.declare_dram_parameter("out", [128, ST, D], F16, isOutput=True)

    with tile.TileContext(nc) as tc, ExitStack() as ctx:
        pool = ctx.enter_context(tc.tile_pool(name="pers", bufs=1))
        mpool = ctx.enter_context(tc.tile_pool(name="mpool", bufs=2 * MPF))
        epool = ctx.enter_context(tc.tile_pool(name="epool", bufs=3))
        rbpool = ctx.enter_context(tc.tile_pool(name="rbpool", bufs=4))
        opool = ctx.enter_context(tc.tile_pool(name="opool", bufs=3))
        scp = ctx.enter_context(tc.tile_pool(name="scp", bufs=2, space="PSUM"))
        tp = ctx.enter_context(tc.tile_pool(name="tp", bufs=1, space="PSUM"))

        xt = pool.tile([128, KD, L], F16)
        wq_sb = pool.tile([128, KD, FPC], F16)
        wk_sb = pool.tile([128, KD, FPC], F16)
        wv_sb = pool.tile([128, KD, FPC], F16)
        wo_sb = pool.tile([128, 2, D], F16)
        bq_sb = pool.tile([128, 2], F32)
        bk_sb = pool.tile([128, 2], F32)
        # DMA issue order follows the dependency order of the first
        # matmuls: KT needs wk + xt chunk k; V needs wv + xt chunk k.
        nc.sync.dma_start(out=wk_sb[:], in_=wk[:])
        nc.sync.dma_start(out=xt[:, 0, :], in_=xT[:, 0, :])
        nc.sync.dma_start(out=wv_sb[:], in_=wv[:])
        for kd in range(1, KD):
            nc.sync.dma_start(out=xt[:, kd, :], in_=xT[:, kd, :])
        nc.sync.dma_start(out=wq_sb[:], in_=wq[:])
        nc.sync.dma_start(out=bk_sb[:], in_=bk[:])
        nc.sync.dma_start(out=bq_sb[:], in_=bq[:])
        nc.sync.dma_start(out=wo_sb[:], in_=wo[:])

        QT = pool.tile([128, 2, L], F16)   # [feat(2x128), l]: Q^T * 0.125
        KT = pool.tile([128, 2, L], F16)
        # Vaug[:, st, h]: even h -> [V_h | 1], odd h -> [1 | V_h]
        Vaug = pool.tile([128, ST, HPC, 128], F16)
        nc.gpsimd.memset(Vaug[:], 1.0)
        outTs = [pool.tile([128, 2, LTW], F16, name=f"outT{i}")
                 for i in range(LT)]

        # ---- producers, emitted in small quanta between attention s-tile
        # ---- iterations so the PE absorbs them while ACT (exp) streams.
        def emit_kt_half(c, ft):
            lsl = slice(c * LTW, (c + 1) * LTW)
            fsl = slice(ft * 128, (ft + 1) * 128)
            ps = scp.tile([128, 2, LTW], F32, tag="sc", name=f"pk{c}_{ft}")
            for kd in range(KD):
                nc.tensor.matmul(ps[:, 0, :], wk_sb[:, kd, fsl],
                                 xt[:, kd, lsl],
                                 start=(kd == 0), stop=(kd == KD - 1))
            nc.vector.scalar_tensor_tensor(
                KT[:, ft, lsl], ps[:, 0, :], 1.0,
                bk_sb[:, ft:ft + 1].to_broadcast((128, LTW)),
                mybir.AluOpType.mult, mybir.AluOpType.add)

        def emit_qt_half(lt, ft):
            lsl = slice(lt * LTW, (lt + 1) * LTW)
            fsl = slice(ft * 128, (ft + 1) * 128)
            ps = scp.tile([128, 2, LTW], F32, tag="sc", name=f"pq{lt}_{ft}")
            for kd in range(KD):
                nc.tensor.matmul(ps[:, 0, :], wq_sb[:, kd, fsl],
                                 xt[:, kd, lsl],
                                 start=(kd == 0), stop=(kd == KD - 1))
            nc.vector.scalar_tensor_tensor(
                QT[:, ft, lsl], ps[:, 0, :], 0.125,
                bq_sb[:, ft:ft + 1].to_broadcast((128, LTW)),
                mybir.AluOpType.mult, mybir.AluOpType.add)

        def emit_v_pair(j):
            # V projection for s-tiles (2j, 2j+1) into one score-slot alloc
            psv = scp.tile([128, 2, LTW], F32, tag="sc", name=f"psv{j}")
            for i in range(2):
                st = 2 * j + i
                ssl = slice(st * STW, (st + 1) * STW)
                for kd in range(KD):
                    nc.tensor.matmul(psv[:, i, :FPC], xt[:, kd, ssl],
                                     wv_sb[:, kd, :],
                                     start=(kd == 0), stop=(kd == KD - 1))
            for i in range(2):
                st = 2 * j + i
                for h in range(HPC):
                    off = 0 if h % 2 == 0 else 64
                    nc.vector.tensor_copy(Vaug[:, st, h, off:off + 64],
                                          psv[:, i, DK * h:DK * (h + 1)])

        def emit_outproj_group(lt8):
            ps3 = scp.tile([128, 2, LTW], F32, tag="sc", name=f"ps3_{lt8}")
            for nf in range(2):
                nsl = slice(nf * 512, (nf + 1) * 512)
                for pair in range(2):
                    nc.tensor.matmul(
                        ps3[:, nf, :],
                        outTs[lt8 // 4][:, pair,
                                        (lt8 % 4) * 128:(lt8 % 4 + 1) * 128],
                        wo_sb[:, pair, nsl],
                        start=(pair == 0), stop=(pair == 1))
            ob = opool.tile([128, D], F16)
            if lt8 % 2 == 0:
                nc.scalar.copy(ob[:], ps3[:])
            else:
                nc.vector.tensor_copy(ob[:], ps3[:])
            nc.gpsimd.dma_start(out=out[:, lt8, :], in_=ob[:])

        # Injection table: work quanta emitted after attention s-tile `st`
        # of l-tile `lt`.  l-tile 0 absorbs the remaining KT chunks (chunk c
        # is consumed starting at st=4c), V s-tile pairs (pair j is consumed
        # at st=2j) and QT for lt 1; later l-tiles absorb the previous
        # l-tile's output projection and the next QT.
        inject = {}
        inject[(0, 0)] = [lambda: emit_kt_half(1, 0)]
        inject[(0, 1)] = [lambda: emit_kt_half(1, 1)]
        inject[(0, 2)] = [lambda: emit_v_pair(2)]
        inject[(0, 3)] = [lambda: emit_v_pair(3)]
        inject[(0, 4)] = [lambda: emit_kt_half(2, 0)]
        inject[(0, 5)] = [lambda: emit_kt_half(2, 1)]
        inject[(0, 6)] = [lambda: emit_v_pair(4)]
        inject[(0, 7)] = [lambda: emit_v_pair(5)]
        inject[(0, 8)] = [lambda: emit_kt_half(3, 0)]
        inject[(0, 9)] = [lambda: emit_kt_half(3, 1)]
        inject[(0, 10)] = [lambda: emit_v_pair(6)]
        inject[(0, 11)] = [lambda: emit_v_pair(7)]
        inject[(0, 12)] = [lambda: emit_qt_half(1, 0)]
        inject[(0, 14)] = [lambda: emit_qt_half(1, 1)]
        for lt in range(1, LT):
            for g in range(4):
                inject[(lt, 2 * g)] = [
                    lambda lt8=4 * (lt - 1) + g: emit_outproj_group(lt8)]
            if lt + 1 < LT:
                inject[(lt, 9)] = [lambda lt=lt: emit_qt_half(lt + 1, 0)]
                inject[(lt, 11)] = [lambda lt=lt: emit_qt_half(lt + 1, 1)]

        # ---- prefix: just enough to start the attention pipeline.
        emit_kt_half(0, 0)
        emit_kt_half(0, 1)
        emit_qt_half(0, 0)
        emit_qt_half(0, 1)
        emit_v_pair(0)
        emit_v_pair(1)

        mk_tiles = {}

        def prefetch_mask(lt, st):
            if st < ST:
                mk = mpool.tile([128, LTW], F16)
                nc.sync.dma_start(out=mk[:], in_=maskT[st, lt])
                mk_tiles[(lt, st)] = mk

        for lt in range(LT):
            lsl = slice(lt * LTW, (lt + 1) * LTW)
            for st in range(MPF):
                prefetch_mask(lt, st)
            Ts = [tp.tile([128, LTW], F32, tag=f"T{h}", name=f"T{h}_{lt}")
                  for h in range(HPC)]
            for st in range(ST):
                prefetch_mask(lt, st + MPF)
                ssl = slice(st * STW, (st + 1) * STW)
                mk = mk_tiles.pop((lt, st))
                Eall = epool.tile([128, 2, 2, LTW], F16)
                for pair in range(2):
                    sc = scp.tile([128, 2, LTW], F32, tag="sc")
                    for i in range(2):
                        nc.tensor.matmul(
                            sc[:, i, :],
                            KT[64 * i:64 * (i + 1), pair, ssl],
                            QT[64 * i:64 * (i + 1), pair, lsl],
                            start=True, stop=True)
                    nc.scalar.activation(Eall[:, pair, :, :], sc[:], Exp)
                # one fused 0/1 mask multiply for both pairs (FD=2048, 2x)
                nc.vector.tensor_mul(
                    Eall[:], Eall[:],
                    mk[:, None, None, :].to_broadcast((128, 2, 2, LTW)))
                # all four aug matmuls back-to-back: one weight-swap drain
                # boundary per s-tile instead of one per pair
                for pair in range(2):
                    for i in range(2):
                        h = 2 * pair + i
                        nc.tensor.matmul(Ts[h][:], Vaug[:, st, h, :],
                                         Eall[:, pair, i, :],
                                         start=(st == 0), stop=(st == ST - 1))
                for fn in inject.get((lt, st), ()):
                    fn()
            for h in range(HPC):
                # reciprocal_approx_fast only works at partition base 0, so
                # route the row sums through lanes 0:64 in both parities.
                pair, i = divmod(h, 2)
                av_sl = slice(64 * i, 64 * (i + 1))        # av lanes
                rs_sl = slice(64 * (1 - i), 64 * (2 - i))  # row-sum lanes
                rb = rbpool.tile([128, LTW], F32)
                if i == 0:   # av 0:64, sums 64:128 -> move sums down first
                    nc.vector.tensor_copy(rb[64:128, :], Ts[h][rs_sl, :])
                    nc.gpsimd.dma_start(out=rb[0:64, :], in_=rb[64:128, :])
                    nc.vector.reciprocal_approx_fast(out=rb[0:64, :],
                                                     in_=rb[0:64, :])
                else:        # sums 0:64 -> recip at base 0, then move up
                    nc.vector.reciprocal_approx_fast(out=rb[0:64, :],
                                                     in_=Ts[h][rs_sl, :])
                    nc.gpsimd.dma_start(out=rb[64:128, :], in_=rb[0:64, :])
                nc.vector.tensor_mul(outTs[lt][av_sl, pair, :],
                                     Ts[h][av_sl, :], rb[av_sl, :])

        # ---------------- output projection tail (last l-tile) -------------
        for lt8 in range(4 * (LT - 1), 4 * LT):
            emit_outproj_group(lt8)

    nc.compile()
    return nc


def _get_nc():
    global _CACHED_NC
    if _CACHED_NC is None:
        _CACHED_NC = _build()
    return _CACHED_NC


def _prep_core_inputs(c, x, mask, Wq, bq, Wk, bk, Wv, Wo):
    b, g = divmod(c, 4)
    cs = slice(g * FPC, (g + 1) * FPC)

    xT = np.ascontiguousarray(
        x[b].T.reshape(KD, 128, L).transpose(1, 0, 2)).astype(np.float16)
    wq_c = np.ascontiguousarray(
        Wq[:, cs].reshape(KD, 128, FPC).transpose(1, 0, 2)).astype(np.float16)
    wk_c = np.ascontiguousarray(
        Wk[:, cs].reshape(KD, 128, FPC).transpose(1, 0, 2)).astype(np.float16)
    wv_c = np.ascontiguousarray(
        Wv[:, cs].reshape(KD, 128, FPC).transpose(1, 0, 2)).astype(np.float16)
    wo_c = np.ascontiguousarray(
        Wo[cs, :].reshape(2, 128, D).transpose(1, 0, 2)).astype(np.float16)
    bq_c = np.ascontiguousarray(
        (bq[cs] * 0.125).reshape(2, 128).T).astype(np.float32)
    bk_c = np.ascontiguousarray(bk[cs].reshape(2, 128).T).astype(np.float32)
    mT = mask[b].astype(np.float16).T  # [S, L]
    maskT = np.ascontiguousarray(
        mT.reshape(ST, 128, LT, LTW).transpose(0, 2, 1, 3))
    return {"xT": xT, "wq": wq_c, "wk": wk_c, "wv": wv_c, "wo": wo_c,
            "bq": bq_c, "bk": bk_c, "maskT": maskT}


def kernel(x, mask, Wq, bq, Wk, bk, Wv, bv, Wo, bo):
    x = np.asarray(x, np.float32)
    mask = np.asarray(mask)
    Wq, bq = np.asarray(Wq, np.float32), np.asarray(bq, np.float32)
    Wk, bk = np.asarray(Wk, np.float32), np.asarray(bk, np.float32)
    Wv, bv = np.asarray(Wv, np.float32), np.asarray(bv, np.float32)
    Wo, bo = np.asarray(Wo, np.float32), np.asarray(bo, np.float32)

    nc = _get_nc()
    in_maps = [_prep_core_inputs(c, x, mask, Wq, bq, Wk, bk, Wv, Wo)
               for c in range(NCORES)]
    res = run_bass_kernel_spmd(nc, in_maps, list(range(NCORES)))

    const_vec = (bv @ Wo + bo).astype(np.float32)  # A rows sum to 1
    outs = []
    for b in range(B):
        acc = np.zeros((L, D), np.float32)
        for g in range(4):
            part = res.results[4 * b + g]["out"]  # [128, 16, 1024] fp16
            acc += part.transpose(1, 0, 2).reshape(L, D).astype(np.float32)
        acc += const_vec
        outs.append(acc)
    return np.stack(outs)


# revision 8
# speedup vs baseline: 1.1429x; 1.1429x over previous
"""Multi-head attention (B=2, L=S=2048, D=1024, H=16) on 8 Trainium2 cores.

Sharding: core c -> batch b = c // 4, head group g = c % 4 (4 heads per core).
W_Q/K/V column-sharded (256 cols per core), W_O row-sharded (256 rows per core);
the 4 partial outputs per batch are summed on the host (plus bias terms).

Per-core pipeline (all big tensors kept transposed so no on-device transposes):
  projections: QT = 0.125*(x Wq + bq)^T, KT = (x Wk + bk)^T (feature-major
    [256, L]); Vaug = [V_h | ones] per head (seq-major, fp16), V bias folded
    out on the host (softmax rows sum to 1 => + bv @ Wo + bo once).
  attention, per (l-tile 512, s-tile 128): S^T = KT^T QT (row-packed pairs of
    heads, K=64); E = exp(S^T) * maskT (ACT exp from PSUM, one fused 0/1 fp16
    mask multiply per s-tile covering both pairs on DVE at 2x); T_h +=
    Vaug_h^T E accumulates BOTH the head output AND its softmax row-sums in
    one full-array matmul (ones columns act as the reducer; even heads get
    [V|1] -> av in rows 0:64, odd heads [1|V] -> av in rows 64:128 so every
    result lands on the lanes the output-projection layout needs). Per
    l-tile: reciprocal_approx_fast on the sum half, DMA lane-swap to the av
    half's partitions, multiply into outT (fp16).
  out-projection: out_partial = outT^T Wo_rows (K=128, accumulate over the
    two 128-row groups).

Software pipelining: the kernel is EXP/PE co-limited, so emission is fully
pipelined: only KT chunk 0 + QT l-chunk 0 + V s-tiles 0-3 are produced up
front (~16us); the remaining KT chunks, V s-tile pairs and QT chunks are
injected in ~1-2us quanta between attention s-tile iterations of l-tile 0,
and each l-tile's output projection is injected into the next l-tile's
s-loop. V/QT/KT/out-proj all borrow the score PSUM slot rotation ("sc" tag)
so the Ts accumulator banks stay live for attention the whole time.

All matmul operands fp16 (1 cyc/row, no packing restrictions); PSUM fp32.
PSUM budget 8 banks = scores 2x2 + T_h 4x1; projections and output-
projection borrow the score slots, so phases overlap without barriers.
"""
from contextlib import ExitStack

import numpy as np

import concourse.bass as bass
import concourse.mybir as mybir
import concourse.tile as tile
from concourse import bacc
from concourse.bass_utils import run_bass_kernel_spmd

F16 = mybir.dt.float16
F32 = mybir.dt.float32

D = 1024          # d_model
H = 16            # heads
DK = 64           # head dim
B, L = 2, 2048
NCORES = 8
HPC = 4           # heads per core
FPC = HPC * DK    # features per core = 256
KD = D // 128     # 8 contraction subtiles for projections
LT, LTW = 4, 512  # l tiles
ST, STW = 16, 128  # s tiles
MPF = 3           # mask DMA prefetch depth
Ident = mybir.ActivationFunctionType.Identity
Exp = mybir.ActivationFunctionType.Exp

_CACHED_NC = None


def _build():
    nc = bacc.Bacc("TRN2", target_bir_lowering=False, debug=False,
                   num_devices=NCORES)
    xT = nc.declare_dram_parameter("xT", [128, KD, L], F16, isOutput=False)
    wq = nc.declare_dram_parameter("wq", [128, KD, FPC], F16, isOutput=False)
    wk = nc.declare_dram_parameter("wk", [128, KD, FPC], F16, isOutput=False)
    wv = nc.declare_dram_parameter("wv", [128, KD, FPC], F16, isOutput=False)
    wo = nc.declare_dram_parameter("wo", [128, 2, D], F16, isOutput=False)
    bq = nc.declare_dram_parameter("bq", [128, 2], F32, isOutput=False)
    bk = nc.declare_dram_parameter("bk", [128, 2], F32, isOutput=False)
    maskT = nc.declare_dram_parameter("maskT", [ST, LT, 128, LTW], F16,
                                      isOutput=False)
    out = nc.declare_dram_parameter("out", [128, ST, D], F16, isOutput=True)

    with tile.TileContext(nc) as tc, ExitStack() as ctx:
        pool = ctx.enter_context(tc.tile_pool(name="pers", bufs=1))
        mpool = ctx.enter_context(tc.tile_pool(name="mpool", bufs=2 * MPF))
        epool = ctx.enter_context(tc.tile_pool(name="epool", bufs=4))
        rbpool = ctx.enter_context(tc.tile_pool(name="rbpool", bufs=4))
        opool = ctx.enter_context(tc.tile_pool(name="opool", bufs=3))
        scp = ctx.enter_context(tc.tile_pool(name="scp", bufs=2, space="PSUM"))
        tp = ctx.enter_context(tc.tile_pool(name="tp", bufs=1, space="PSUM"))

        xt = pool.tile([128, KD, L], F16)
        wq_sb = pool.tile([128, KD, FPC], F16)
        wk_sb = pool.tile([128, KD, FPC], F16)
        wv_sb = pool.tile([128, KD, FPC], F16)
        wo_sb = pool.tile([128, 2, D], F16)
        bq_sb = pool.tile([128, 2], F32)
        bk_sb = pool.tile([128, 2], F32)
        # DMA issue order follows the dependency order of the first
        # matmuls: KT needs wk + xt chunk k; V needs wv + xt chunk k.
        nc.sync.dma_start(out=wk_sb[:], in_=wk[:])
        nc.sync.dma_start(out=xt[:, 0, :], in_=xT[:, 0, :])
        nc.sync.dma_start(out=wv_sb[:], in_=wv[:])
        for kd in range(1, KD):
            nc.sync.dma_start(out=xt[:, kd, :], in_=xT[:, kd, :])
        nc.sync.dma_start(out=wq_sb[:], in_=wq[:])
        nc.sync.dma_start(out=bk_sb[:], in_=bk[:])
        nc.sync.dma_start(out=bq_sb[:], in_=bq[:])
        nc.sync.dma_start(out=wo_sb[:], in_=wo[:])

        QT = pool.tile([128, 2, L], F16)   # [feat(2x128), l]: Q^T * 0.125
        KT = pool.tile([128, 2, L], F16)
        # Vaug[:, st, h]: even h -> [V_h | 1], odd h -> [1 | V_h]
        Vaug = pool.tile([128, ST, HPC, 128], F16)
        nc.gpsimd.memset(Vaug[:], 1.0)
        outTs = [pool.tile([128, 2, LTW], F16, name=f"outT{i}")
                 for i in range(LT)]

        # ---- producers, emitted in small quanta between attention s-tile
        # ---- iterations so the PE absorbs them while ACT (exp) streams.
        def emit_kt_half(c, ft):
            lsl = slice(c * LTW, (c + 1) * LTW)
            fsl = slice(ft * 128, (ft + 1) * 128)
            ps = scp.tile([128, 2, LTW], F32, tag="sc", name=f"pk{c}_{ft}")
            for kd in range(KD):
                nc.tensor.matmul(ps[:, 0, :], wk_sb[:, kd, fsl],
                                 xt[:, kd, lsl],
                                 start=(kd == 0), stop=(kd == KD - 1))
            nc.vector.scalar_tensor_tensor(
                KT[:, ft, lsl], ps[:, 0, :], 1.0,
                bk_sb[:, ft:ft + 1].to_broadcast((128, LTW)),
                mybir.AluOpType.mult, mybir.AluOpType.add)

        def emit_qt_half(lt, ft):
            lsl = slice(lt * LTW, (lt + 1) * LTW)
            fsl = slice(ft * 128, (ft + 1) * 128)
            ps = scp.tile([128, 2, LTW], F32, tag="sc", name=f"pq{lt}_{ft}")
            for kd in range(KD):
                nc.tensor.matmul(ps[:, 0, :], wq_sb[:, kd, fsl],
                                 xt[:, kd, lsl],
                                 start=(kd == 0), stop=(kd == KD - 1))
            nc.vector.scalar_tensor_tensor(
                QT[:, ft, lsl], ps[:, 0, :], 0.125,
                bq_sb[:, ft:ft + 1].to_broadcast((128, LTW)),
                mybir.AluOpType.mult, mybir.AluOpType.add)

        def emit_v_pair(j):
            # V projection for s-tiles (2j, 2j+1) into one score-slot alloc
            psv = scp.tile([128, 2, LTW], F32, tag="sc", name=f"psv{j}")
            for i in range(2):
                st = 2 * j + i
                ssl = slice(st * STW, (st + 1) * STW)
                for kd in range(KD):
                    nc.tensor.matmul(psv[:, i, :FPC], xt[:, kd, ssl],
                                     wv_sb[:, kd, :],
                                     start=(kd == 0), stop=(kd == KD - 1))
            for i in range(2):
                st = 2 * j + i
                for h in range(HPC):
                    off = 0 if h % 2 == 0 else 64
                    nc.vector.tensor_copy(Vaug[:, st, h, off:off + 64],
                                          psv[:, i, DK * h:DK * (h + 1)])

        def emit_outproj_group(lt8):
            ps3 = scp.tile([128, 2, LTW], F32, tag="sc", name=f"ps3_{lt8}")
            for nf in range(2):
                nsl = slice(nf * 512, (nf + 1) * 512)
                for pair in range(2):
                    nc.tensor.matmul(
                        ps3[:, nf, :],
                        outTs[lt8 // 4][:, pair,
                                        (lt8 % 4) * 128:(lt8 % 4 + 1) * 128],
                        wo_sb[:, pair, nsl],
                        start=(pair == 0), stop=(pair == 1))
            ob = opool.tile([128, D], F16)
            if lt8 % 2 == 0:
                nc.scalar.copy(ob[:], ps3[:])
            else:
                nc.vector.tensor_copy(ob[:], ps3[:])
            nc.gpsimd.dma_start(out=out[:, lt8, :], in_=ob[:])

        # Injection table: work quanta emitted after attention s-tile `st`
        # of l-tile `lt`.  l-tile 0 absorbs the remaining KT chunks (chunk c
        # is consumed starting at st=4c), V s-tile pairs (pair j is consumed
        # at st=2j) and QT for lt 1; later l-tiles absorb the previous
        # l-tile's output projection and the next QT.
        inject = {}
        inject[(0, 0)] = [lambda: emit_kt_half(1, 0)]
        inject[(0, 1)] = [lambda: emit_kt_half(1, 1)]
        inject[(0, 2)] = [lambda: emit_v_pair(2)]
        inject[(0, 3)] = [lambda: emit_v_pair(3)]
        inject[(0, 4)] = [lambda: emit_kt_half(2, 0)]
        inject[(0, 5)] = [lambda: emit_kt_half(2, 1)]
        inject[(0, 6)] = [lambda: emit_v_pair(4)]
        inject[(0, 7)] = [lambda: emit_v_pair(5)]
        inject[(0, 8)] = [lambda: emit_kt_half(3, 0)]
        inject[(0, 9)] = [lambda: emit_kt_half(3, 1)]
        inject[(0, 10)] = [lambda: emit_v_pair(6)]
        inject[(0, 11)] = [lambda: emit_v_pair(7)]
        inject[(0, 12)] = [lambda: emit_qt_half(1, 0)]
        inject[(0, 14)] = [lambda: emit_qt_half(1, 1)]
        for lt in range(1, LT):
            for g in range(4):
                inject[(lt, 2 * g)] = [
                    lambda lt8=4 * (lt - 1) + g: emit_outproj_group(lt8)]
            if lt + 1 < LT:
                inject[(lt, 9)] = [lambda lt=lt: emit_qt_half(lt + 1, 0)]
                inject[(lt, 11)] = [lambda lt=lt: emit_qt_half(lt + 1, 1)]

        # ---- prefix: just enough to start the attention pipeline.
        emit_kt_half(0, 0)
        emit_kt_half(0, 1)
        emit_qt_half(0, 0)
        emit_qt_half(0, 1)
        emit_v_pair(0)
        emit_v_pair(1)

        mk_tiles = {}
        mk_order = [(lt, st) for lt in range(LT) for st in range(ST)]

        def prefetch_mask(pos):
            if pos < len(mk_order):
                plt, pst = mk_order[pos]
                mk = mpool.tile([128, LTW], F16)
                nc.sync.dma_start(out=mk[:], in_=maskT[pst, plt])
                mk_tiles[(plt, pst)] = mk

        for pos in range(MPF):
            prefetch_mask(pos)

        for lt in range(LT):
            lsl = slice(lt * LTW, (lt + 1) * LTW)
            Ts = [tp.tile([128, LTW], F32, tag=f"T{h}", name=f"T{h}_{lt}")
                  for h in range(HPC)]
            for st in range(ST):
                prefetch_mask(lt * ST + st + MPF)
                ssl = slice(st * STW, (st + 1) * STW)
                mk = mk_tiles.pop((lt, st))
                Es = []
                for pair in range(2):
                    sc = scp.tile([128, 2, LTW], F32, tag="sc")
                    for i in range(2):
                        nc.tensor.matmul(
                            sc[:, i, :],
                            KT[64 * i:64 * (i + 1), pair, ssl],
                            QT[64 * i:64 * (i + 1), pair, lsl],
                            start=True, stop=True)
                    E = epool.tile([128, 2, LTW], F16, name=f"E{pair}")
                    nc.scalar.activation(E[:], sc[:], Exp)
                    nc.vector.tensor_mul(
                        E[:], E[:],
                        mk[:, None, :].to_broadcast((128, 2, LTW)))
                    Es.append(E)
                # injected producer quanta go here: in PE program order they
                # sit between the score and aug matmuls, filling the PE wait
                # for the exp+mask chain of this s-tile.
                for fn in inject.get((lt, st), ()):
                    fn()
                # all four aug matmuls back-to-back: one weight-swap drain
                # boundary per s-tile instead of one per pair.  At st==0 the
                # odd heads go first: their T banks are released first by the
                # previous l-tile's phase-batched normalization.
                h_order = (1, 3, 0, 2) if st == 0 else (0, 1, 2, 3)
                for h in h_order:
                    pair, i = divmod(h, 2)
                    nc.tensor.matmul(Ts[h][:], Vaug[:, st, h, :],
                                     Es[pair][:, i, :],
                                     start=(st == 0), stop=(st == ST - 1))
            # Per-head softmax normalization.  reciprocal_approx_fast only
            # works at partition base 0, so row sums are routed through
            # lanes 0:64 in both parities; the cross-partition moves go
            # through the Sync DMA queue.  Phase-batched so the odd heads
            # (whose sums already sit at base 0) release their T banks
            # ~3us before the even heads, unblocking the next l-tile's
            # first aug matmuls early.
            rbs = [rbpool.tile([128, LTW], F32, name=f"rb{h}")
                   for h in range(HPC)]
            # odd heads (sums at 0:64): recip immediately
            for h in (1, 3):
                nc.vector.reciprocal_approx_fast(out=rbs[h][0:64, :],
                                                 in_=Ts[h][0:64, :])
            # even heads (sums at 64:128): stage for the lane swap
            for h in (0, 2):
                nc.vector.tensor_copy(rbs[h][64:128, :], Ts[h][64:128, :])
            for h in (1, 3):
                nc.sync.dma_start(out=rbs[h][64:128, :], in_=rbs[h][0:64, :])
            for h in (0, 2):
                nc.sync.dma_start(out=rbs[h][0:64, :], in_=rbs[h][64:128, :])
            # odd heads finish first (av lanes 64:128)
            for h in (1, 3):
                nc.vector.tensor_mul(outTs[lt][64:128, h // 2, :],
                                     Ts[h][64:128, :], rbs[h][64:128, :])
            for h in (0, 2):
                nc.vector.reciprocal_approx_fast(out=rbs[h][0:64, :],
                                                 in_=rbs[h][0:64, :])
                nc.vector.tensor_mul(outTs[lt][0:64, h // 2, :],
                                     Ts[h][0:64, :], rbs[h][0:64, :])

        # ---------------- output projection tail (last l-tile) -------------
        for lt8 in range(4 * (LT - 1), 4 * LT):
            emit_outproj_group(lt8)

    nc.compile()
    return nc


def _get_nc():
    global _CACHED_NC
    if _CACHED_NC is None:
        _CACHED_NC = _build()
    return _CACHED_NC


def _prep_core_inputs(c, x, mask, Wq, bq, Wk, bk, Wv, Wo):
    b, g = divmod(c, 4)
    cs = slice(g * FPC, (g + 1) * FPC)

    xT = np.ascontiguousarray(
        x[b].T.reshape(KD, 128, L).transpose(1, 0, 2)).astype(np.float16)
    wq_c = np.ascontiguousarray(
        Wq[:, cs].reshape(KD, 128, FPC).transpose(1, 0, 2)).astype(np.float16)
    wk_c = np.ascontiguousarray(
        Wk[:, cs].reshape(KD, 128, FPC).transpose(1, 0, 2)).astype(np.float16)
    wv_c = np.ascontiguousarray(
        Wv[:, cs].reshape(KD, 128, FPC).transpose(1, 0, 2)).astype(np.float16)
    wo_c = np.ascontiguousarray(
        Wo[cs, :].reshape(2, 128, D).transpose(1, 0, 2)).astype(np.float16)
    bq_c = np.ascontiguousarray(
        (bq[cs] * 0.125).reshape(2, 128).T).astype(np.float32)
    bk_c = np.ascontiguousarray(bk[cs].reshape(2, 128).T).astype(np.float32)
    mT = mask[b].astype(np.float16).T  # [S, L]
    maskT = np.ascontiguousarray(
        mT.reshape(ST, 128, LT, LTW).transpose(0, 2, 1, 3))
    return {"xT": xT, "wq": wq_c, "wk": wk_c, "wv": wv_c, "wo": wo_c,
            "bq": bq_c, "bk": bk_c, "maskT": maskT}


def kernel(x, mask, Wq, bq, Wk, bk, Wv, bv, Wo, bo):
    x = np.asarray(x, np.float32)
    mask = np.asarray(mask)
    Wq, bq = np.asarray(Wq, np.float32), np.asarray(bq, np.float32)
    Wk, bk = np.asarray(Wk, np.float32), np.asarray(bk, np.float32)
    Wv, bv = np.asarray(Wv, np.float32), np.asarray(bv, np.float32)
    Wo, bo = np.asarray(Wo, np.float32), np.asarray(bo, np.float32)

    nc = _get_nc()
    in_maps = [_prep_core_inputs(c, x, mask, Wq, bq, Wk, bk, Wv, Wo)
               for c in range(NCORES)]
    res = run_bass_kernel_spmd(nc, in_maps, list(range(NCORES)))

    const_vec = (bv @ Wo + bo).astype(np.float32)  # A rows sum to 1
    outs = []
    for b in range(B):
        acc = np.zeros((L, D), np.float32)
        for g in range(4):
            part = res.results[4 * b + g]["out"]  # [128, 16, 1024] fp16
            acc += part.transpose(1, 0, 2).reshape(L, D).astype(np.float32)
        acc += const_vec
        outs.append(acc)
    return np.stack(outs)


# revision 13
# speedup vs baseline: 1.1851x; 1.0369x over previous
"""Multi-head attention (B=2, L=S=2048, D=1024, H=16) on 8 Trainium2 cores.

Sharding: core c -> batch b = c // 4, head group g = c % 4 (4 heads per core).
W_Q/K/V column-sharded (256 cols per core), W_O row-sharded (256 rows per core);
the 4 partial outputs per batch are summed on the host (plus bias terms).

Per-core pipeline (all big tensors kept transposed so no on-device transposes):
  projections: QT = 0.125*(x Wq + bq)^T, KT = (x Wk + bk)^T (feature-major
    [256, L]); Vaug = [V_h | ones] per head (seq-major, fp16), V bias folded
    out on the host (softmax rows sum to 1 => + bv @ Wo + bo once).
  attention, per (l-tile 512, s-tile 128): S^T = KT^T QT (row-packed pairs of
    heads, K=64); E = exp(S^T) * maskT (ACT exp from PSUM, one fused 0/1 fp16
    mask multiply per s-tile covering both pairs on DVE at 2x); T_h +=
    Vaug_h^T E accumulates BOTH the head output AND its softmax row-sums in
    one full-array matmul (ones columns act as the reducer; even heads get
    [V|1] -> av in rows 0:64, odd heads [1|V] -> av in rows 64:128 so every
    result lands on the lanes the output-projection layout needs). Per
    l-tile: reciprocal_approx_fast on the sum half, DMA lane-swap to the av
    half's partitions, multiply into outT (fp16).
  out-projection: out_partial = outT^T Wo_rows (K=128, accumulate over the
    two 128-row groups).

Software pipelining: the kernel is EXP/PE co-limited, so emission is fully
pipelined: only KT chunk 0 + QT l-chunk 0 + V s-tiles 0-3 are produced up
front (~16us); the remaining KT chunks, V s-tile pairs and QT chunks are
injected in ~1-2us quanta between attention s-tile iterations of l-tile 0,
and each l-tile's output projection is injected into the next l-tile's
s-loop. V/QT/KT/out-proj all borrow the score PSUM slot rotation ("sc" tag)
so the Ts accumulator banks stay live for attention the whole time.

All matmul operands fp16 (1 cyc/row, no packing restrictions); PSUM fp32.
PSUM budget 8 banks = scores 2x2 + T_h 4x1; projections and output-
projection borrow the score slots, so phases overlap without barriers.
"""
from contextlib import ExitStack

import numpy as np

import concourse.bass as bass
import concourse.mybir as mybir
import concourse.tile as tile
from concourse import bacc
from concourse.bass_utils import run_bass_kernel_spmd

F16 = mybir.dt.float16
F32 = mybir.dt.float32

D = 1024          # d_model
H = 16            # heads
DK = 64           # head dim
B, L = 2, 2048
NCORES = 8
HPC = 4           # heads per core
FPC = HPC * DK    # features per core = 256
KD = D // 128     # 8 contraction subtiles for projections
LT, LTW = 4, 512  # l tiles
ST, STW = 16, 128  # s tiles
MPF = 3           # mask DMA prefetch depth
Ident = mybir.ActivationFunctionType.Identity
Exp = mybir.ActivationFunctionType.Exp

_CACHED_NC = None


def _build():
    nc = bacc.Bacc("TRN2", target_bir_lowering=False, debug=False,
                   num_devices=NCORES)
    xT = nc.declare_dram_parameter("xT", [128, KD, L], F16, isOutput=False)
    wq = nc.declare_dram_parameter("wq", [128, KD, FPC], F16, isOutput=False)
    wk = nc.declare_dram_parameter("wk", [128, KD, FPC], F16, isOutput=False)
    wv = nc.declare_dram_parameter("wv", [128, KD, FPC], F16, isOutput=False)
    wo = nc.declare_dram_parameter("wo", [128, 2, D], F16, isOutput=False)
    bq = nc.declare_dram_parameter("bq", [128, 2], F32, isOutput=False)
    bk = nc.declare_dram_parameter("bk", [128, 2], F32, isOutput=False)
    maskT = nc.declare_dram_parameter("maskT", [ST, LT, 128, LTW], F16,
                                      isOutput=False)
    out = nc.declare_dram_parameter("out", [128, ST, D], F16, isOutput=True)

    with tile.TileContext(nc) as tc, ExitStack() as ctx:
        pool = ctx.enter_context(tc.tile_pool(name="pers", bufs=1))
        mpool = ctx.enter_context(tc.tile_pool(name="mpool", bufs=2 * MPF))
        epool = ctx.enter_context(tc.tile_pool(name="epool", bufs=4))
        rbpool = ctx.enter_context(tc.tile_pool(name="rbpool", bufs=1))
        avpool = ctx.enter_context(tc.tile_pool(name="avpool", bufs=1))
        opool = ctx.enter_context(tc.tile_pool(name="opool", bufs=3))
        scp = ctx.enter_context(tc.tile_pool(name="scp", bufs=2, space="PSUM"))
        tp = ctx.enter_context(tc.tile_pool(name="tp", bufs=1, space="PSUM"))

        xt = pool.tile([128, KD, L], F16)
        wq_sb = pool.tile([128, KD, FPC], F16)
        wk_sb = pool.tile([128, KD, FPC], F16)
        wv_sb = pool.tile([128, KD, FPC], F16)
        wo_sb = pool.tile([128, 2, D], F16)
        bq_sb = pool.tile([128, 2], F32)
        bk_sb = pool.tile([128, 2], F32)
        # DMA issue order follows the dependency order of the first
        # matmuls.  x is transferred in l-quarters: the whole prefix (KT
        # chunk 0, QT l-chunk 0, V s-tiles 0-3) only touches sequence
        # positions 0:512, so the PE can start ~3.5us in instead of
        # waiting ~12us for all of x.
        nc.sync.dma_start(out=wk_sb[:], in_=wk[:])
        for kd in range(KD):
            nc.sync.dma_start(out=xt[:, kd, 0:LTW], in_=xT[:, kd, 0:LTW])
        nc.sync.dma_start(out=wv_sb[:], in_=wv[:])
        nc.sync.dma_start(out=wq_sb[:], in_=wq[:])
        nc.sync.dma_start(out=bk_sb[:], in_=bk[:])
        nc.sync.dma_start(out=bq_sb[:], in_=bq[:])
        for q in range(1, LT):
            qsl = slice(q * LTW, (q + 1) * LTW)
            for kd in range(KD):
                nc.sync.dma_start(out=xt[:, kd, qsl], in_=xT[:, kd, qsl])
        nc.sync.dma_start(out=wo_sb[:], in_=wo[:])

        QT = pool.tile([128, 2, L], F16)   # [feat(2x128), l]: Q^T * 0.125
        KT = pool.tile([128, 2, L], F16)
        # Vaug[:, st, h]: even h -> [V_h | 1], odd h -> [1 | V_h]
        Vaug = pool.tile([128, ST, HPC, 128], F16)
        nc.gpsimd.memset(Vaug[:], 1.0)
        outTs = [pool.tile([128, 2, LTW], F16, name=f"outT{i}")
                 for i in range(LT)]

        # ---- producers, emitted in small quanta between attention s-tile
        # ---- iterations so the PE absorbs them while ACT (exp) streams.
        def emit_kt_half(c, ft):
            lsl = slice(c * LTW, (c + 1) * LTW)
            fsl = slice(ft * 128, (ft + 1) * 128)
            ps = scp.tile([128, 2, LTW], F32, tag="sc", name=f"pk{c}_{ft}")
            for kd in range(KD):
                nc.tensor.matmul(ps[:, 0, :], wk_sb[:, kd, fsl],
                                 xt[:, kd, lsl],
                                 start=(kd == 0), stop=(kd == KD - 1))
            nc.vector.scalar_tensor_tensor(
                KT[:, ft, lsl], ps[:, 0, :], 1.0,
                bk_sb[:, ft:ft + 1].to_broadcast((128, LTW)),
                mybir.AluOpType.mult, mybir.AluOpType.add)

        def emit_qt_half(lt, ft):
            lsl = slice(lt * LTW, (lt + 1) * LTW)
            fsl = slice(ft * 128, (ft + 1) * 128)
            ps = scp.tile([128, 2, LTW], F32, tag="sc", name=f"pq{lt}_{ft}")
            for kd in range(KD):
                nc.tensor.matmul(ps[:, 0, :], wq_sb[:, kd, fsl],
                                 xt[:, kd, lsl],
                                 start=(kd == 0), stop=(kd == KD - 1))
            nc.vector.scalar_tensor_tensor(
                QT[:, ft, lsl], ps[:, 0, :], 0.125,
                bq_sb[:, ft:ft + 1].to_broadcast((128, LTW)),
                mybir.AluOpType.mult, mybir.AluOpType.add)

        def emit_v_pair(j):
            # V projection for s-tiles (2j, 2j+1) into one score-slot alloc
            psv = scp.tile([128, 2, LTW], F32, tag="sc", name=f"psv{j}")
            for i in range(2):
                st = 2 * j + i
                ssl = slice(st * STW, (st + 1) * STW)
                for kd in range(KD):
                    nc.tensor.matmul(psv[:, i, :FPC], xt[:, kd, ssl],
                                     wv_sb[:, kd, :],
                                     start=(kd == 0), stop=(kd == KD - 1))
            for i in range(2):
                st = 2 * j + i
                for h in range(HPC):
                    off = 0 if h % 2 == 0 else 64
                    nc.vector.tensor_copy(Vaug[:, st, h, off:off + 64],
                                          psv[:, i, DK * h:DK * (h + 1)])

        def emit_outproj_group(lt8):
            ps3 = scp.tile([128, 2, LTW], F32, tag="sc", name=f"ps3_{lt8}")
            for nf in range(2):
                nsl = slice(nf * 512, (nf + 1) * 512)
                for pair in range(2):
                    nc.tensor.matmul(
                        ps3[:, nf, :],
                        outTs[lt8 // 4][:, pair,
                                        (lt8 % 4) * 128:(lt8 % 4 + 1) * 128],
                        wo_sb[:, pair, nsl],
                        start=(pair == 0), stop=(pair == 1))
            ob = opool.tile([128, D], F16)
            # DVE only: a scalar.copy here would stall the ACT exp stream
            nc.vector.tensor_copy(ob[:], ps3[:])
            nc.gpsimd.dma_start(out=out[:, lt8, :], in_=ob[:])

        # Injection table: work quanta emitted after attention s-tile `st`
        # of l-tile `lt`.  l-tile 0 absorbs the remaining KT chunks (chunk c
        # is consumed starting at st=4c), V s-tile pairs (pair j is consumed
        # at st=2j) and QT for lt 1; later l-tiles absorb the previous
        # l-tile's output projection and the next QT.
        inject = {}
        inject[(0, 0)] = [lambda: emit_kt_half(1, 0)]
        inject[(0, 1)] = [lambda: emit_kt_half(1, 1)]
        inject[(0, 2)] = [lambda: emit_v_pair(2)]
        inject[(0, 3)] = [lambda: emit_v_pair(3)]
        inject[(0, 4)] = [lambda: emit_kt_half(2, 0)]
        inject[(0, 5)] = [lambda: emit_kt_half(2, 1)]
        inject[(0, 6)] = [lambda: emit_v_pair(4)]
        inject[(0, 7)] = [lambda: emit_v_pair(5)]
        inject[(0, 8)] = [lambda: emit_kt_half(3, 0)]
        inject[(0, 9)] = [lambda: emit_kt_half(3, 1)]
        inject[(0, 10)] = [lambda: emit_v_pair(6)]
        inject[(0, 11)] = [lambda: emit_v_pair(7)]
        inject[(0, 12)] = [lambda: emit_qt_half(1, 0)]
        inject[(0, 14)] = [lambda: emit_qt_half(1, 1)]
        for lt in range(1, LT):
            for g in range(4):
                inject[(lt, 3 * g + 3)] = [
                    lambda lt8=4 * (lt - 1) + g: emit_outproj_group(lt8)]
            if lt + 1 < LT:
                inject[(lt, 8)] = [lambda lt=lt: emit_qt_half(lt + 1, 0)]
                inject[(lt, 10)] = [lambda lt=lt: emit_qt_half(lt + 1, 1)]

        # ---- prefix: just enough to start the attention pipeline.
        emit_kt_half(0, 0)
        emit_kt_half(0, 1)
        emit_qt_half(0, 0)
        emit_qt_half(0, 1)
        emit_v_pair(0)
        emit_v_pair(1)

        mk_tiles = {}
        mk_order = [(lt, st) for lt in range(LT) for st in range(ST)]

        def prefetch_mask(pos):
            if pos < len(mk_order):
                plt, pst = mk_order[pos]
                mk = mpool.tile([128, LTW], F16)
                nc.sync.dma_start(out=mk[:], in_=maskT[pst, plt])
                mk_tiles[(plt, pst)] = mk

        for pos in range(MPF):
            prefetch_mask(pos)

        for lt in range(LT):
            lsl = slice(lt * LTW, (lt + 1) * LTW)
            Ts = [tp.tile([128, LTW], F32, tag=f"T{h}", name=f"T{h}_{lt}")
                  for h in range(HPC)]
            for st in range(ST):
                prefetch_mask(lt * ST + st + MPF)
                ssl = slice(st * STW, (st + 1) * STW)
                mk = mk_tiles.pop((lt, st))
                Es = []
                for pair in range(2):
                    sc = scp.tile([128, 2, LTW], F32, tag="sc")
                    for i in range(2):
                        nc.tensor.matmul(
                            sc[:, i, :],
                            KT[64 * i:64 * (i + 1), pair, ssl],
                            QT[64 * i:64 * (i + 1), pair, lsl],
                            start=True, stop=True)
                    E = epool.tile([128, 2, LTW], F16, name=f"E{pair}")
                    nc.scalar.activation(E[:], sc[:], Exp)
                    nc.vector.tensor_mul(
                        E[:], E[:],
                        mk[:, None, :].to_broadcast((128, 2, LTW)))
                    Es.append(E)
                # injected producer quanta go here: in PE program order they
                # sit between the score and aug matmuls, filling the PE wait
                # for the exp+mask chain of this s-tile.
                for fn in inject.get((lt, st), ()):
                    fn()
                # all four aug matmuls back-to-back: one weight-swap drain
                # boundary per s-tile instead of one per pair.  At st==0 the
                # odd heads go first: their T banks are released first by the
                # previous l-tile's phase-batched normalization.
                h_order = (1, 3, 0, 2) if st == 0 else (0, 1, 2, 3)
                for h in h_order:
                    pair, i = divmod(h, 2)
                    nc.tensor.matmul(Ts[h][:], Vaug[:, st, h, :],
                                     Es[pair][:, i, :],
                                     start=(st == 0), stop=(st == ST - 1))
            # Per-head softmax normalization.  The T PSUM banks are on the
            # critical path (the next l-tile's aug matmuls wait on them), so
            # each head's two halves are pulled out of PSUM as early as
            # possible — av halves via ACT (idle at the boundary), row sums
            # via DVE recip (odd heads, already at base 0) or copy (even
            # heads) — releasing the banks long before the lane swaps,
            # reciprocals and final multiplies complete on SBUF staging.
            rbs = [rbpool.tile([128, LTW], F32, name=f"rb{h}")
                   for h in range(HPC)]
            avs = [avpool.tile([128, LTW], F32, name=f"av{h}")
                   for h in range(HPC)]
            for h in (1, 3):   # odd: av at 64:128, sums at 0:64
                nc.scalar.copy(avs[h][64:128, :], Ts[h][64:128, :])
                nc.vector.reciprocal_approx_fast(out=rbs[h][0:64, :],
                                                 in_=Ts[h][0:64, :])
            for h in (0, 2):   # even: av at 0:64, sums at 64:128
                nc.scalar.copy(avs[h][0:64, :], Ts[h][0:64, :])
                nc.vector.tensor_copy(rbs[h][64:128, :], Ts[h][64:128, :])
            for h in (1, 3):
                nc.sync.dma_start(out=rbs[h][64:128, :], in_=rbs[h][0:64, :])
            for h in (0, 2):
                nc.sync.dma_start(out=rbs[h][0:64, :], in_=rbs[h][64:128, :])
            for h in (1, 3):
                nc.vector.tensor_mul(outTs[lt][64:128, h // 2, :],
                                     avs[h][64:128, :], rbs[h][64:128, :])
            for h in (0, 2):
                nc.vector.reciprocal_approx_fast(out=rbs[h][0:64, :],
                                                 in_=rbs[h][0:64, :])
                nc.vector.tensor_mul(outTs[lt][0:64, h // 2, :],
                                     avs[h][0:64, :], rbs[h][0:64, :])

        # ---------------- output projection tail (last l-tile) -------------
        for lt8 in range(4 * (LT - 1), 4 * LT):
            emit_outproj_group(lt8)

    nc.compile()
    return nc


def _get_nc():
    global _CACHED_NC
    if _CACHED_NC is None:
        _CACHED_NC = _build()
    return _CACHED_NC


def _prep_core_inputs(c, x, mask, Wq, bq, Wk, bk, Wv, Wo):
    b, g = divmod(c, 4)
    cs = slice(g * FPC, (g + 1) * FPC)

    xT = np.ascontiguousarray(
        x[b].T.reshape(KD, 128, L).transpose(1, 0, 2)).astype(np.float16)
    wq_c = np.ascontiguousarray(
        Wq[:, cs].reshape(KD, 128, FPC).transpose(1, 0, 2)).astype(np.float16)
    wk_c = np.ascontiguousarray(
        Wk[:, cs].reshape(KD, 128, FPC).transpose(1, 0, 2)).astype(np.float16)
    wv_c = np.ascontiguousarray(
        Wv[:, cs].reshape(KD, 128, FPC).transpose(1, 0, 2)).astype(np.float16)
    wo_c = np.ascontiguousarray(
        Wo[cs, :].reshape(2, 128, D).transpose(1, 0, 2)).astype(np.float16)
    bq_c = np.ascontiguousarray(
        (bq[cs] * 0.125).reshape(2, 128).T).astype(np.float32)
    bk_c = np.ascontiguousarray(bk[cs].reshape(2, 128).T).astype(np.float32)
    mT = mask[b].astype(np.float16).T  # [S, L]
    maskT = np.ascontiguousarray(
        mT.reshape(ST, 128, LT, LTW).transpose(0, 2, 1, 3))
    return {"xT": xT, "wq": wq_c, "wk": wk_c, "wv": wv_c, "wo": wo_c,
            "bq": bq_c, "bk": bk_c, "maskT": maskT}


def kernel(x, mask, Wq, bq, Wk, bk, Wv, bv, Wo, bo):
    x = np.asarray(x, np.float32)
    mask = np.asarray(mask)
    Wq, bq = np.asarray(Wq, np.float32), np.asarray(bq, np.float32)
    Wk, bk = np.asarray(Wk, np.float32), np.asarray(bk, np.float32)
    Wv, bv = np.asarray(Wv, np.float32), np.asarray(bv, np.float32)
    Wo, bo = np.asarray(Wo, np.float32), np.asarray(bo, np.float32)

    nc = _get_nc()
    in_maps = [_prep_core_inputs(c, x, mask, Wq, bq, Wk, bk, Wv, Wo)
               for c in range(NCORES)]
    res = run_bass_kernel_spmd(nc, in_maps, list(range(NCORES)))

    const_vec = (bv @ Wo + bo).astype(np.float32)  # A rows sum to 1
    outs = []
    for b in range(B):
        acc = np.zeros((L, D), np.float32)
        for g in range(4):
            part = res.results[4 * b + g]["out"]  # [128, 16, 1024] fp16
            acc += part.transpose(1, 0, 2).reshape(L, D).astype(np.float32)
        acc += const_vec
        outs.append(acc)
    return np.stack(outs)
